# revision 8
# baseline (speedup 1.0000x reference)
"""TRN2 Bass kernel v3 for nn_Aggregator — core-count parameterized.

res[n] = sum_{e: head_e == n} all_emb[tail_e] * weight[edge_type_e]

Final config: 4 cores, 4 SWDGE queues, 3-deep gather buffer, 6-deep input
slabs, 24-row relation one-hot. v3 changes vs v2:
  - N_CORES parameterizable (1/2/4/8): tail/delta/rel streams are loaded in
    per-chunk slabs (rotating SBUF slots) instead of fully preloaded, so SBUF
    stays bounded for any core count.
  - rel one-hot DRAM layout is [NCHUNK*32, CHE]; each chunk's 32-row slab is
    DMA'd into a rotating 32-partition band of one [128, CHE] SBUF tensor
    (matmul lhsT base partition must be 0/32/64/96).
  - out writes batched: one dma_start per chunk ([128,4,C] -> 512 rows).
"""

import sys

if "/opt/trn_rl_repo" not in sys.path:
    sys.path.insert(0, "/opt/trn_rl_repo")

import numpy as np

# ---- problem constants (hardcoded per contract) ----
N_NODES = 50000
N_EDGES = 600000
N_REL = 24
C = 128

N_CORES = 4
NPC = N_NODES // N_CORES
HALF = N_NODES // 2           # emb table split point (int16 gather indices)
NB = {8: 56, 4: 112, 2: 224, 1: 448}[N_CORES]   # node blocks per core
TA = 6                        # low-half tiles per block
T = 2 * TA                    # 12 tiles per block
HCAP = TA * 128               # 768 edges per (block, half)
TILES = NB * T
BPC = 4                       # blocks per chunk
TPC = BPC * T                 # 48 tiles per chunk
NCHUNK = NB // BPC
E_PAD = TILES * 128
ECOLS = E_PAD // 16           # wrapped idx columns
CHE = TPC * 128               # edge slots per chunk (6144)
HCOLS = (TPC // 2) * 8        # idx cols per half-phase (192)
NSLOT = 6                     # input-slab rotation depth
EGS = 3                       # emb_g gather-buffer rotation depth

_CACHE = {}


def _build_bass():
    from concourse import bacc, mybir

    bf16 = mybir.dt.bfloat16
    f32 = mybir.dt.float32

    nc = bacc.Bacc(
        "TRN2",
        target_bir_lowering=False,
        debug=False,
        num_devices=N_CORES,
        num_swdge_queues=4,
    )
    emb = nc.dram_tensor("emb", [N_NODES, C], bf16, kind="ExternalInput")
    wdr = nc.dram_tensor("wdr", [128, C], bf16, kind="ExternalInput")
    tail_i = nc.dram_tensor("tail_i", [128, ECOLS], mybir.dt.int16, kind="ExternalInput")
    delta_i = nc.dram_tensor("delta_i", [128, 2 * TILES], bf16, kind="ExternalInput")
    rel_i = nc.dram_tensor("rel_i", [NCHUNK * N_REL, CHE], bf16, kind="ExternalInput")
    iota_i = nc.dram_tensor("iota_i", [128, 128], bf16, kind="ExternalInput")
    out = nc.dram_tensor("out", [NB * 128, C], f32, kind="ExternalOutput")

    from contextlib import ExitStack

    with ExitStack() as stack:
        block = stack.enter_context(nc.Block())
        tail_sb = stack.enter_context(
            nc.sbuf_tensor("tail_sb", [128, NSLOT, 2 * HCOLS], mybir.dt.int16)
        )
        delta_sb = stack.enter_context(
            nc.sbuf_tensor("delta_sb", [128, NSLOT, 2 * TPC], bf16)
        )
        rel_sb = stack.enter_context(nc.sbuf_tensor("rel_sb", [128, CHE], bf16))
        iota_sb = stack.enter_context(nc.sbuf_tensor("iota_sb", [128, 128], bf16))
        w_sb = stack.enter_context(nc.sbuf_tensor("w_sb", [128, C], bf16))
        emb_g = stack.enter_context(nc.sbuf_tensor("emb_g", [128, EGS, TPC, C], bf16))
        wsel_sb = stack.enter_context(
            nc.sbuf_tensor("wsel_sb", [128, 2, TPC, C], bf16)
        )
        msg = stack.enter_context(nc.sbuf_tensor("msg", [128, 2, TPC, C], bf16))
        oh_sb = stack.enter_context(nc.sbuf_tensor("oh_sb", [128, 2, TPC, C], bf16))
        outsb = stack.enter_context(nc.sbuf_tensor("outsb", [128, 2, 4, C], f32))
        tsems = [stack.enter_context(nc.semaphore(f"ts{i}")) for i in range(NSLOT)]
        dsems = [stack.enter_context(nc.semaphore(f"ds{i}")) for i in range(NSLOT)]
        rsems = [stack.enter_context(nc.semaphore(f"rs{i}")) for i in range(3)]
        ism = stack.enter_context(nc.semaphore("ism"))
        wsm = stack.enter_context(nc.semaphore("wsm"))
        # 4-ary gather sems (indexed ch%4): with the 3-deep emb_g buffer,
        # gathers of chunk ch+2 can already be issued while DVE still waits on
        # chunk ch's — a parity (2-ary) sem would make the wait threshold
        # satisfiable by a mix of both chunks' completions. ch%4 spacing plus
        # the gpsimd msem back-pressure keeps exactly one chunk-group in
        # flight per sem. Sem index parity matches the queue (sem locked to
        # one SWDGE queue): lows on queue ch%2, highs on 2+ch%2.
        gls = [stack.enter_context(nc.semaphore(f"gl{i}")) for i in range(4)]
        ghs = [stack.enter_context(nc.semaphore(f"gh{i}")) for i in range(4)]
        osem = stack.enter_context(nc.semaphore("osem"))    # one-hot built (1/chunk)
        msem = stack.enter_context(nc.semaphore("msem"))    # msg mult (1/q)
        pwsem = stack.enter_context(nc.semaphore("pwsem"))  # PE wsel group (1/q)
        pmsem = stack.enter_context(nc.semaphore("pmsem"))  # PE main chunk (1/chunk)
        asem = stack.enter_context(nc.semaphore("asem"))    # Act wsel copy (1/q)
        csem = stack.enter_context(nc.semaphore("csem"))    # Act out copy (1/chunk)
        wsems = [stack.enter_context(nc.semaphore(f"ws{i}")) for i in range(2)]
        psum = nc.alloc_psum_tensor("psum", [128, 8, 512], f32)

        # psum wsel slot for within-chunk tile t=12q+i: parity on quarter q
        def psum_wsel(q, i):
            return psum[:, 3 * (q % 2) + i // 4, 128 * (i % 4) : 128 * (i % 4) + 128]

        # psum block accumulators: all 4 blocks of chunk x share bank 6+(x%2)
        def psum_blk(x, q):
            return psum[:, 6 + (x % 2), 128 * q : 128 * q + 128]

        @block.sync
        def _(sync):
            sync.dma_start(out=w_sb[:], in_=wdr[:]).then_inc(wsm, 16)
            sync.dma_start(out=iota_sb[:], in_=iota_i[:]).then_inc(ism, 16)
            for ch in range(NCHUNK):
                s4 = ch % NSLOT
                # tail slab: slot used by chunk ch-NSLOT is free once its msg
                # mults ran (msem is an engine sem, so no DMA-completion
                # reordering ambiguity; it also implies the gathers landed)
                if ch >= NSLOT:
                    sync.wait_ge(msem, 4 * (ch - NSLOT + 1))
                sync.dma_start(
                    out=tail_sb[:, s4, :],
                    in_=tail_i[:, ch * 2 * HCOLS : (ch + 1) * 2 * HCOLS],
                ).then_inc(tsems[s4], 16)
                # delta slab: consumer is DVE one-hot (osem 1/chunk)
                if ch >= NSLOT:
                    sync.wait_ge(osem, ch - NSLOT + 1)
                sync.dma_start(
                    out=delta_sb[:, s4, :],
                    in_=delta_i[:, ch * 2 * TPC : (ch + 1) * 2 * TPC],
                ).then_inc(dsems[s4], 16)
                # rel slab -> partition band (ch%3)*32 (matmul base partition
                # must be 0/32/64); consumer is PE wsel (pwsem 4/chunk)
                if ch >= 3:
                    sync.wait_ge(pwsem, 4 * (ch - 2))
                band = 32 * (ch % 3)
                sync.dma_start(
                    out=rel_sb[band : band + N_REL, :],
                    in_=rel_i[ch * N_REL : (ch + 1) * N_REL, :],
                ).then_inc(rsems[ch % 3], 16)
                # out-writes interleaved so the csem wait never deadlocks
                # against the slab-load back-pressure above
                x = ch - NSLOT
                if x >= 0:
                    sync.wait_ge(csem, x + 1)
                    sync.dma_start(
                        out=out[x * 512 : (x + 1) * 512, :].rearrange(
                            "(q p) c -> p q c", p=128
                        ),
                        in_=outsb[:, x % 2, :, :],
                    ).then_inc(wsems[x % 2], 16)
            for x in range(max(0, NCHUNK - NSLOT), NCHUNK):
                sync.wait_ge(csem, x + 1)
                sync.dma_start(
                    out=out[x * 512 : (x + 1) * 512, :].rearrange(
                        "(q p) c -> p q c", p=128
                    ),
                    in_=outsb[:, x % 2, :, :],
                ).then_inc(wsems[x % 2], 16)

        @block.gpsimd
        def _(gpsimd):
            for ch in range(NCHUNK):
                sl = ch % 2
                s4 = ch % NSLOT
                gpsimd.wait_ge(tsems[s4], 16 * (ch // NSLOT + 1))
                if ch >= EGS:
                    # emb_g slot (3-deep) free: all 4 msg mults of ch-3 done
                    gpsimd.wait_ge(msem, 4 * (ch - 2))
                sg = ch % EGS
                s4g = ch % 4
                for u in range(3):      # HW limit: 1024 idxs per dma_gather
                    gpsimd.dma_gather(
                        out_ap=emb_g[:, sg, 8 * u : 8 * (u + 1), :],
                        in_ap=emb[0:HALF, :],
                        idxs_ap=tail_sb[:, s4, 64 * u : 64 * (u + 1)],
                        num_idxs=1024,
                        num_idxs_reg=1024,
                        elem_size=C,
                        queue_num=sl,
                    ).then_inc(gls[s4g], 16)
                for u in range(3):
                    gpsimd.dma_gather(
                        out_ap=emb_g[:, sg, 24 + 8 * u : 24 + 8 * (u + 1), :],
                        in_ap=emb[HALF:N_NODES, :],
                        idxs_ap=tail_sb[:, s4, HCOLS + 64 * u : HCOLS + 64 * (u + 1)],
                        num_idxs=1024,
                        num_idxs_reg=1024,
                        elem_size=C,
                        queue_num=2 + sl,
                    ).then_inc(ghs[s4g], 16)

        @block.vector
        def _(vector):
            from concourse import mybir as mb

            vector.wait_ge(ism, 16)
            for ch in range(NCHUNK):
                sl = ch % 2
                s4 = ch % NSLOT
                vector.wait_ge(dsems[s4], 16 * (ch // NSLOT + 1))
                # oh/msg slot free: PE main of chunk ch-2 done
                if ch >= 2:
                    vector.wait_ge(pmsem, ch - 1)
                d4 = (
                    delta_sb[:, s4, :]
                    .rearrange("p (t k) -> p t k", k=2)
                    .unsqueeze(2)
                    .to_broadcast([128, TPC, 64, 2])
                )
                i4 = (
                    iota_sb[:]
                    .rearrange("p (j k) -> p j k", k=2)
                    .unsqueeze(1)
                    .to_broadcast([128, TPC, 64, 2])
                )
                o4 = oh_sb[:, sl, :, :].rearrange("p t (j k) -> p t j k", k=2)
                vector.tensor_tensor(
                    out=o4, in0=d4, in1=i4, op=mb.AluOpType.is_equal
                ).then_inc(osem, 1)
                for q in range(4):
                    g = 4 * ch + q
                    vector.wait_ge(asem, g + 1)
                    vector.wait_ge(
                        gls[ch % 4] if q < 2 else ghs[ch % 4], 48 * (ch // 4 + 1)
                    )
                    vector.tensor_mul(
                        out=msg[:, sl, 12 * q : 12 * (q + 1), :],
                        in0=emb_g[:, ch % EGS, 12 * q : 12 * (q + 1), :],
                        in1=wsel_sb[:, sl, 12 * q : 12 * (q + 1), :],
                    ).then_inc(msem, 1)

        @block.scalar
        def _(scalar):
            from concourse import mybir as mb

            for ch in range(NCHUNK):
                sl = ch % 2
                for q in range(4):
                    g = 4 * ch + q
                    scalar.wait_ge(pwsem, g + 1)
                    if g >= 8:
                        scalar.wait_ge(msem, g - 7)
                    pv = psum[:, 3 * (q % 2) : 3 * (q % 2) + 3, :].rearrange(
                        "p b (s c) -> p b s c", c=C
                    )
                    wv = wsel_sb[:, sl, 12 * q : 12 * (q + 1), :].rearrange(
                        "p (b s) c -> p b s c", s=4
                    )
                    scalar.activation(
                        out=wv, in_=pv, func=mb.ActivationFunctionType.Copy
                    ).then_inc(asem, 1)
                if ch >= 1:
                    x = ch - 1
                    scalar.wait_ge(pmsem, x + 1)
                    if x >= 2:
                        scalar.wait_ge(wsems[x % 2], 16 * (x // 2))
                    scalar.activation(
                        out=outsb[:, x % 2, :, :],
                        in_=psum[:, 6 + (x % 2), :].rearrange("p (q c) -> p q c", c=C),
                        func=mb.ActivationFunctionType.Copy,
                    ).then_inc(csem, 1)
            x = NCHUNK - 1
            scalar.wait_ge(pmsem, x + 1)
            scalar.wait_ge(wsems[x % 2], 16 * (x // 2))
            scalar.activation(
                out=outsb[:, x % 2, :, :],
                in_=psum[:, 6 + (x % 2), :].rearrange("p (q c) -> p q c", c=C),
                func=mb.ActivationFunctionType.Copy,
            ).then_inc(csem, 1)

        @block.tensor
        def _(tensor):
            tensor.wait_ge(wsm, 16)

            def mains(x):
                sl = x % 2
                tensor.wait_ge(osem, x + 1)
                if x >= 2:
                    tensor.wait_ge(csem, x - 1)
                first = True
                for qb in range(BPC):          # block-major order
                    for half in range(2):
                        for bt in range(TA):
                            j = half * (TPC // 2) + qb * TA + bt
                            need = 4 * x + (1 if qb < 2 else 2) + (2 if half else 0)
                            tensor.wait_ge(msem, need)
                            last = qb == BPC - 1 and half == 1 and bt == TA - 1
                            mm = tensor.matmul(
                                out=psum_blk(x, qb),
                                lhsT=oh_sb[:, sl, j, :],
                                rhs=msg[:, sl, j, :],
                                start=first,
                                stop=last,
                            )
                            first = False
                            if last:
                                mm.then_inc(pmsem, 1)

            for ch in range(NCHUNK):
                band = 32 * (ch % 3)
                tensor.wait_ge(rsems[ch % 3], 16 * (ch // 3 + 1))
                for q in range(4):
                    g = 4 * ch + q
                    if g >= 2:
                        tensor.wait_ge(asem, g - 1)
                    for i in range(12):
                        t = 12 * q + i
                        mm = tensor.matmul(
                            out=psum_wsel(q, i),
                            lhsT=rel_sb[band : band + N_REL, 128 * t : 128 * (t + 1)],
                            rhs=w_sb[band : band + N_REL, :],
                            start=(i % 4 == 0),
                            stop=(i % 4 == 3),
                        )
                        if i == 11:
                            mm.then_inc(pwsem, 1)
                if ch >= 1:
                    mains(ch - 1)
            mains(NCHUNK - 1)

    nc.finalize()
    return nc


def _pack_blocks(dA, dB):
    """Greedy pair-balanced packing of NPC nodes into NB blocks."""
    dT = dA + dB
    order = np.argsort(-dT, kind="stable")
    loadA = np.zeros(NB, np.int64)
    loadB = np.zeros(NB, np.int64)
    cnt = np.zeros(NB, np.int64)
    blk_of = np.empty(NPC, np.int64)
    slot_of = np.empty(NPC, np.int64)
    for n in order:
        a, bV = dA[n], dB[n]
        score = np.maximum(loadA + a, loadB + bV).astype(np.float64)
        feas = (cnt < 128) & (loadA + a <= HCAP) & (loadB + bV <= HCAP)
        if not feas.any():
            raise AssertionError("block packing failed: no feasible block")
        score[~feas] = np.inf
        b = int(np.argmin(score + cnt * 1e-6))
        blk_of[n] = b
        slot_of[n] = cnt[b]
        loadA[b] += a
        loadB[b] += bV
        cnt[b] += 1
    return blk_of, slot_of


def _prep_core(head_local, tail, etype, bf16):
    """Index-only host prep for one core.

    Returns (tail16 [128,ECOLS] i16, delta2x [128,2*TILES] bf16,
             rel1h [NCHUNK*32, CHE] bf16, row_of_node [NPC])."""
    nE = head_local.shape[0]
    assert nE <= E_PAD, f"core edge count {nE} exceeds capacity {E_PAD}"
    hi = (tail >= HALF).astype(np.int64)
    dA = np.bincount(head_local[hi == 0], minlength=NPC)
    dB = np.bincount(head_local[hi == 1], minlength=NPC)
    blk_of, slot_of = _pack_blocks(dA, dB)

    b_e = blk_of[head_local]
    d_e = slot_of[head_local]
    key = b_e * 2 + hi
    perm = np.argsort(key, kind="stable")
    skey = key[perm]
    cnt = np.bincount(skey, minlength=2 * NB)
    assert cnt.max() <= HCAP, f"(block,half) overflow: {cnt.max()} > {HCAP}"
    gstart = np.concatenate([[0], np.cumsum(cnt)[:-1]])
    rank = np.arange(nE) - np.repeat(gstart, cnt)

    sb = skey // 2
    sh = skey % 2
    chunk = sb // BPC
    q = sb % BPC
    tile0 = chunk * TPC + sh * (TPC // 2) + q * TA
    pos = tile0 * 128 + rank

    tail_pad = np.zeros(E_PAD, np.int16)
    et_pad = np.full(E_PAD, 99, np.int64)       # 99 -> all-zero rel1h column
    delta_pad = np.zeros(E_PAD, np.float32)
    stail = tail[perm]
    tail_pad[pos] = np.where(stail >= HALF, stail - HALF, stail).astype(np.int16)
    et_pad[pos] = etype[perm]
    delta_pad[pos] = d_e[perm]

    tail16 = np.ascontiguousarray(np.tile(tail_pad.reshape(ECOLS, 16).T, (8, 1)))

    delta2 = delta_pad.reshape(TILES, 128).T            # [128, TILES]
    delta2x = np.repeat(delta2, 2, axis=1).astype(bf16)  # [128, 2*TILES]

    et_c = et_pad.reshape(NCHUNK, CHE)
    rr = np.arange(N_REL)
    rel = (et_c[:, None, :] == rr[None, :, None]).reshape(NCHUNK * N_REL, CHE)
    rel1h = rel.astype(bf16)

    row_of_node = blk_of * 128 + slot_of
    return tail16, delta2x, rel1h, row_of_node


def _prep_all(all_emb, edge_index, edge_type, weight):
    from concourse import mybir

    bf16 = mybir.dt.np(mybir.dt.bfloat16)

    all_emb = np.asarray(all_emb, dtype=np.float32)
    weight = np.asarray(weight, dtype=np.float32)
    head = np.asarray(edge_index[0]).astype(np.int64)
    tail = np.asarray(edge_index[1]).astype(np.int64)
    etype = np.asarray(edge_type).astype(np.int64)

    emb_b = np.ascontiguousarray(all_emb.astype(bf16))
    wpad = np.zeros((32, C), np.float32)
    wpad[:N_REL] = weight
    w_b = np.ascontiguousarray(np.tile(wpad, (4, 1))).astype(bf16)
    iota = np.broadcast_to(np.arange(128, dtype=np.float32), (128, 128)).astype(bf16)

    core_of = head // NPC
    in_maps = []
    rows = []
    for k in range(N_CORES):
        m = core_of == k
        t16, d2x, r1h, row_of = _prep_core(head[m] - k * NPC, tail[m], etype[m], bf16)
        in_maps.append(
            {
                "emb": emb_b,
                "wdr": w_b,
                "tail_i": t16,
                "delta_i": d2x,
                "rel_i": r1h,
                "iota_i": iota,
            }
        )
        rows.append(row_of)
    return in_maps, rows


class _Runner:
    """Compile once, execute many (same as baseline runner)."""

    def __init__(self):
        import jax
        from jax.sharding import Mesh, PartitionSpec
        from jax.experimental.shard_map import shard_map
        from concourse import mybir
        from concourse.bass2jax import (
            _bass_exec_p,
            install_neuronx_cc_hook,
            partition_id_tensor,
        )

        install_neuronx_cc_hook()
        nc = _build_bass()
        self.nc = nc

        partition_name = (
            nc.partition_id_tensor.name if nc.partition_id_tensor else None
        )
        in_names, out_names, out_avals = [], [], []
        for alloc in nc.m.functions[0].allocations:
            if not isinstance(alloc, mybir.MemoryLocationSet):
                continue
            name = alloc.memorylocations[0].name
            if alloc.kind == "ExternalInput":
                if name != partition_name:
                    in_names.append(name)
            elif alloc.kind == "ExternalOutput":
                out_names.append(name)
                out_avals.append(
                    jax.core.ShapedArray(
                        tuple(alloc.tensor_shape), mybir.dt.np(alloc.dtype)
                    )
                )
        self.in_names = list(in_names)
        self.out_names = list(out_names)
        self.out_shapes = [tuple(a.shape) for a in out_avals]
        self.out_dtypes = [a.dtype for a in out_avals]
        n_params = len(in_names)
        all_names = in_names + out_names
        if partition_name is not None:
            all_names = all_names + [partition_name]

        def _body(*args):
            operands = list(args)
            if partition_name is not None:
                operands.append(partition_id_tensor())
            outs = _bass_exec_p.bind(
                *operands,
                out_avals=tuple(out_avals),
                in_names=tuple(all_names),
                out_names=tuple(out_names),
                lowering_input_output_aliases=(),
                sim_require_finite=True,
                sim_require_nnan=True,
                nc=nc,
            )
            return tuple(outs)

        devices = jax.devices()[:N_CORES]
        assert len(devices) >= 1
        mesh = Mesh(np.asarray(devices), ("core",))
        nio = n_params + len(out_names)
        self._fn = jax.jit(
            shard_map(
                _body,
                mesh=mesh,
                in_specs=(PartitionSpec("core"),) * nio,
                out_specs=(PartitionSpec("core"),) * len(out_names),
                check_rep=False,
            ),
            donate_argnums=tuple(range(n_params, nio)),
            keep_unused=True,
        )
        self._jax = jax
        self._sharding = jax.sharding.NamedSharding(mesh, PartitionSpec("core"))
        self._zeros_np = [
            np.zeros((N_CORES * s[0], *s[1:]), d)
            for s, d in zip(self.out_shapes, self.out_dtypes)
        ]

    def stage_inputs(self, in_maps):
        jax = self._jax
        concat = [
            np.concatenate([np.asarray(m[n]) for m in in_maps], axis=0)
            for n in self.in_names
        ]
        return [jax.device_put(a, self._sharding) for a in concat]

    def stage_zeros(self):
        return [self._jax.device_put(z, self._sharding) for z in self._zeros_np]

    def execute(self, staged_inputs, staged_zeros):
        outs = self._fn(*staged_inputs, *staged_zeros)
        self._jax.block_until_ready(outs)
        return outs

    def run(self, in_maps):
        outs = self.execute(self.stage_inputs(in_maps), self.stage_zeros())
        res = []
        for c in range(N_CORES):
            res.append(
                {
                    n: np.asarray(outs[i]).reshape(N_CORES, *self.out_shapes[i])[c]
                    for i, n in enumerate(self.out_names)
                }
            )
        return res


def _get_runner():
    if "runner" not in _CACHE:
        _CACHE["runner"] = _Runner()
    return _CACHE["runner"]


def kernel(all_emb, edge_index, edge_type, weight):
    in_maps, rows = _prep_all(all_emb, edge_index, edge_type, weight)
    results = _get_runner().run(in_maps)

    out = np.empty((N_NODES, C), np.float32)
    for k in range(N_CORES):
        blk = np.asarray(results[k]["out"]).reshape(NB * 128, C)
        out[k * NPC : (k + 1) * NPC] = blk[rows[k]]
    return out
